# revision 21
# baseline (speedup 1.0000x reference)
"""Trainium2 kernel for nn_ContrastiveLoss (N=4096, D=1024), SPMD over 8 NeuronCores.

Strategy (2x4 core grid, fp8 DoubleRow matmuls at the PE roofline):
  - Host: l2-normalize back_VF/back_AF in f64, scale by 16 and quantize to
    e4m3, pre-transpose into DoubleRow-blocked layouts, compute diag sims
    and the pre-feature cosine term (both O(N*D), same class as the
    normalization already done here).
  - Core (rg, cg) of a 2x4 grid computes its [2048, 1024] tile of
    E = exp(Vn @ An^T):
      * TensorE: 16 groups x 8 fp8 DoubleRow matmuls (K=256 each) into a
        [128, 1024] PSUM pair; short HAM-warmup matmuls first so the clock
        ramp overlaps the initial DMA wait
      * ScalarE: exp(PSUM / 256) with fused row-sum (accum_out)
      * VectorE: f32 column-partial adds; the last group is split into
        512-col halves so the exp->add->store tail is half as deep
      * DMA scheduling (trace-derived): each HWDGE queue retires ~90
        descriptors/us no matter the line size, so every dma_start moves
        128 lines in ~1.4us — use the fattest lines possible; sync's
        queue starts ~1us before scalar's; same-tile DMAs from two
        engines serialize (coarse WAW), so vn is split into two SBUF
        tiles homed on different engines.
    Outputs per core: rowsum partials [128, 17] (last chunk split in
    halves), column partials [128, 1024] bf16 (partition-summed on host).
  - Host: O(N) final assembly (log/ratio/sums) in f64.
"""

import os
import sys

import numpy as np

for _p in ("/opt/trn_rl_repo",):
    if _p not in sys.path and os.path.isdir(_p):
        sys.path.insert(0, _p)

N = 4096
D = 1024
NCORES = 8
RG = 2                   # row groups
CG = 4                   # col groups
ROWS = N // RG           # 2048 rows per core
COLS = N // CG           # 1024 cols per core
MCH = ROWS // 128        # 16 row chunks per core
MCL = MCH // 2           # 8 chunks per vn half-tile
KCH = D // 128           # 8 contraction chunks of 128
KD2 = KCH // 2           # 4 DoubleRow chunks of 256
NB = 512                 # matmul moving free dim (one PSUM bank)
NBL = COLS // NB         # 2 column blocks per core

MARGIN = 0.2
BALANCE = 0.5
BIAS = 1.0
EPS = 1e-18

FP8_SCALE = 16.0  # host pre-scale so e4m3 keeps the values out of subnormals

_CACHE = {}
LAST_RESULT = None  # BassKernelResults of the most recent run (for test harness)


def _build_nc():
    import concourse.bass as bass  # noqa: F401
    import concourse.bacc as bacc
    import concourse.tile as tile
    from concourse import mybir
    from contextlib import ExitStack

    BF16 = mybir.dt.bfloat16
    F32 = mybir.dt.float32
    FP8 = mybir.dt.float8e4
    Exp = mybir.ActivationFunctionType.Exp
    DoubleRow = mybir.MatmulPerfMode.DoubleRow

    nc = bacc.Bacc("TRN2", debug=False, num_devices=NCORES)

    MCW = 1024  # vnT columns per row chunk
    HALF = MCL * MCW  # 8192 columns per vn half

    # DRAM I/O (per core). Layouts chosen so every DMA is contiguous.
    # vnT[p, mc*1024 + k2*256 + i*128 + dm] = Vn_slab[mc*128 + dm, (2*k2+i)*128 + p] * FP8_SCALE
    vnT_d = nc.dram_tensor("vnT", [128, MCH * MCW], FP8, kind="ExternalInput")
    # anT[n, p, k2*2*NB + i*NB + c] = An_slab[n*NB + c, (2*k2+i)*128 + p] * FP8_SCALE
    anT_d = nc.dram_tensor("anT", [NBL, 128, KCH * NB], FP8, kind="ExternalInput")

    # rowsum[p, mc] partials; the last chunk is split into columns MCH-1, MCH
    rowsum_d = nc.dram_tensor("rowsum", [128, MCH + 1], F32, kind="ExternalOutput")
    # colp[p, j] = sum over mc of exp chunk [mc][p, j]  (host sums partitions)
    colp_d = nc.dram_tensor("colp", [128, COLS], BF16, kind="ExternalOutput")

    with tile.TileContext(nc) as tc:
        with ExitStack() as ctx:
            singles = ctx.enter_context(tc.tile_pool(name="singles", bufs=1))

            vn_t0 = singles.tile([128, 2 * MCW], FP8, tag="vn_t0")
            vn_t1 = singles.tile([128, HALF - 2 * MCW], FP8, tag="vn_t1")
            vn_hi = singles.tile([128, HALF], FP8, tag="vn_hi")
            an_sb = []
            for n in range(NBL):
                an_t = singles.tile([128, KCH * NB], FP8, tag=f"an{n}")
                an_sb.append(an_t)

            # DMA schedule. Each dma_start moves 128 partition lines in
            # ~1.4us of queue time regardless of line size, sync's queue
            # starts ~1us before scalar's, and same-tile DMAs must stay on
            # one engine. So: everything the stream needs first (vn mc0-1,
            # an0, an1) goes on sync as three whole-tile DMAs (~10.1, ~11.5,
            # ~12.9); the rest of vn rides scalar's queue with slack.
            nc.sync.dma_start(vn_t0[:], vnT_d.ap()[:, 0 : 2 * MCW])
            nc.scalar.dma_start(vn_t1[:], vnT_d.ap()[:, 2 * MCW : HALF])
            nc.sync.dma_start(an_sb[0][:], anT_d.ap()[0])
            nc.scalar.dma_start(vn_hi[:], vnT_d.ap()[:, HALF : 2 * HALF])
            nc.sync.dma_start(an_sb[1][:], anT_d.ap()[1])

            efold = singles.tile([128, COLS], F32, tag="efold")
            colp = singles.tile([128, COLS], BF16, tag="colp")
            rs = singles.tile([128, MCH + 1], F32, tag="rs")
            ones_b = singles.tile([128, 1], BF16, tag="ones_b")
            nc.vector.memset(ones_b[:], 1.0)
            dummy = singles.tile([128, 256], BF16, tag="dummy")
            nc.vector.memset(dummy[:], 0.0)

            psum = ctx.enter_context(tc.tile_pool(name="mm_psum", bufs=3, space="PSUM"))
            wup = ctx.enter_context(tc.tile_pool(name="wup_psum", bufs=1, space="PSUM"))
            epool = ctx.enter_context(tc.tile_pool(name="etile", bufs=3))

            # HAM warmup: keep TensorE busy during the initial DMA wait so
            # the clock ramp starts as early as possible.
            wps = wup.tile([128, 256], mybir.dt.float32, tag="wup")
            NWARM = 16
            for i in range(NWARM):
                nc.tensor.matmul(
                    wps[0:1, :], ones_b[:], dummy[:],
                    start=(i == 0), stop=(i == NWARM - 1),
                )

            # Main stream: 16 groups of 8 DoubleRow matmuls. Half-outer:
            # group 0's first four matmuls touch only an0, buying an1 an
            # extra ~1.7us of DMA time; the last group's halves retire
            # ~0.85us apart, shortening the exp->add->store tail.
            descale = 1.0 / (FP8_SCALE * FP8_SCALE)
            for mc in range(MCH):
                if mc < 2:
                    vtile, voff = vn_t0, mc * MCW
                elif mc < MCL:
                    vtile, voff = vn_t1, (mc - 2) * MCW
                else:
                    vtile, voff = vn_hi, (mc - MCL) * MCW
                ps = psum.tile([128, 2 * NB], mybir.dt.float32)
                for half in range(NBL):
                    a_t = an_sb[half]
                    for k2 in range(KD2):
                        w3 = (
                            vtile[:, voff + k2 * 256 : voff + (k2 + 1) * 256]
                            .rearrange("p (i m) -> p i m", i=2)
                        )
                        a3 = (
                            a_t[:, k2 * 2 * NB : (k2 + 1) * 2 * NB]
                            .rearrange("p (i c) -> p i c", i=2)
                        )
                        nc.tensor.matmul(
                            ps[:, half * NB : (half + 1) * NB],
                            w3,
                            a3,
                            start=(k2 == 0),
                            stop=(k2 == KD2 - 1),
                            perf_mode=DoubleRow,
                        )
                if mc == 0:
                    nc.scalar.activation(
                        efold[:], ps[:], Exp, scale=descale,
                        accum_out=rs[:, mc : mc + 1],
                    )
                elif mc == MCH - 1:
                    # Last chunk in halves: half the serial exp->add->store
                    # tail. Row-sum halves land in rs cols MCH-1 / MCH
                    # (host adds); per-half colp DMA overlaps the other
                    # half's compute.
                    et = epool.tile([128, 2 * NB], F32)
                    for h in range(2):
                        hs = slice(h * NB, (h + 1) * NB)
                        nc.scalar.activation(
                            et[:, hs], ps[:, hs], Exp, scale=descale,
                            accum_out=rs[:, mc + h : mc + h + 1],
                        )
                        nc.vector.tensor_add(colp[:, hs], efold[:, hs], et[:, hs])
                        nc.sync.dma_start(colp_d.ap()[:, hs], colp[:, hs])
                    # rowsum rides scalar's otherwise-idle queue, right
                    # after its last accumulator read
                    nc.scalar.dma_start(rowsum_d.ap(), rs[:])
                else:
                    et = epool.tile([128, 2 * NB], F32)
                    nc.scalar.activation(
                        et[:], ps[:], Exp, scale=descale,
                        accum_out=rs[:, mc : mc + 1],
                    )
                    nc.vector.tensor_add(efold[:], efold[:], et[:])

    nc.compile()
    return nc


def _get_nc():
    if "nc" not in _CACHE:
        _CACHE["nc"] = _build_nc()
    return _CACHE["nc"]


def _prep_inputs(pre_VF, pre_AF, back_VF, back_AF):
    """Normalize + quantize + relayout on host; returns per-core in_maps,
    host diag, and the host pre-cosine term."""
    import ml_dtypes

    V = np.asarray(back_VF, dtype=np.float64)
    A = np.asarray(back_AF, dtype=np.float64)
    Vn = V / np.sqrt((V * V).sum(-1, keepdims=True) + EPS)
    An = A / np.sqrt((A * A).sum(-1, keepdims=True) + EPS)
    diag = np.einsum("ij,ij->i", Vn, An)

    pV = np.asarray(pre_VF, dtype=np.float64)
    pA = np.asarray(pre_AF, dtype=np.float64)
    pVn = pV / np.sqrt((pV * pV).sum(-1, keepdims=True) + EPS)
    pAn = pA / np.sqrt((pA * pA).sum(-1, keepdims=True) + EPS)
    pre_cos = np.einsum("ij,ij->i", pVn, pAn)

    fp8 = ml_dtypes.float8_e4m3
    Vn8 = (Vn * FP8_SCALE).astype(fp8)
    An8 = (An * FP8_SCALE).astype(fp8)

    # vnT[p, mc*1024 + k2*256 + i*128 + dm] = Vn8_slab[mc*128 + dm, (2*k2+i)*128 + p]
    vnTs = []
    for rg in range(RG):
        sl = Vn8[rg * ROWS : (rg + 1) * ROWS]
        vnTs.append(
            np.ascontiguousarray(
                sl.reshape(MCH, 128, KD2, 2, 128)  # [mc, dm, k2, i, p]
                .transpose(4, 0, 2, 3, 1)          # [p, mc, k2, i, dm]
                .reshape(128, MCH * KD2 * 2 * 128)
            )
        )

    # anT[n, p, k2*2*NB + i*NB + c] = An8_slab[n*NB + c, (2*k2+i)*128 + p]
    anTs = []
    for cg in range(CG):
        sl = An8[cg * COLS : (cg + 1) * COLS]
        anTs.append(
            np.ascontiguousarray(
                sl.reshape(NBL, NB, KD2, 2, 128)   # [n, c, k2, i, p]
                .transpose(0, 4, 2, 3, 1)          # [n, p, k2, i, c]
                .reshape(NBL, 128, KCH * NB)
            )
        )

    in_maps = []
    for c in range(NCORES):
        rg, cg = c // CG, c % CG
        in_maps.append({"vnT": vnTs[rg], "anT": anTs[cg]})
    return in_maps, diag, pre_cos


def _assemble(outs, diag, pre_cos):
    """O(N) final reduction on host, f64."""
    rowsum = np.zeros(N, dtype=np.float64)
    colsum = np.zeros(N, dtype=np.float64)
    for c in range(NCORES):
        rg, cg = c // CG, c % CG
        # rowsum[p, mc]: row = rg*ROWS + mc*128 + p, partial over this cg;
        # the last chunk's sum is split across columns MCH-1 and MCH
        rsc = outs[c]["rowsum"].astype(np.float64)  # [128, MCH + 1]
        rsc = np.concatenate(
            [rsc[:, : MCH - 1], (rsc[:, MCH - 1] + rsc[:, MCH])[:, None]], axis=1
        )
        rowsum[rg * ROWS : (rg + 1) * ROWS] += rsc.T.reshape(ROWS)
        colsum[cg * COLS : (cg + 1) * COLS] += (
            outs[c]["colp"].astype(np.float64).sum(axis=0)
        )

    dE = np.exp(diag)
    pos = np.exp(diag - MARGIN)
    neg_V = rowsum - dE
    neg_A = colsum - dE
    L_V = np.log(pos / (pos + neg_V)).sum()
    L_A = np.log(pos / (pos + neg_A)).sum()
    L_pre = pre_cos.sum()

    loss = BALANCE * (-1.0 / BIAS) * (L_V + L_A) + (1.0 - BALANCE) * L_pre
    return np.array(loss, dtype=np.float32)


def kernel(pre_VF, pre_AF, back_VF, back_AF):
    global LAST_RESULT
    from concourse import bass_utils

    nc = _get_nc()
    in_maps, diag, pre_cos = _prep_inputs(pre_VF, pre_AF, back_VF, back_AF)
    res = bass_utils.run_bass_kernel_spmd(nc, in_maps, core_ids=list(range(NCORES)))
    LAST_RESULT = res
    return _assemble(res.results, diag, pre_cos)


# revision 22
# speedup vs baseline: 1.2031x; 1.2031x over previous
"""Trainium2 kernel for nn_ContrastiveLoss (N=4096, D=1024), SPMD over 8 NeuronCores.

Strategy (2x4 core grid, fp8 DoubleRow matmuls at the PE roofline):
  - Host: l2-normalize back_VF/back_AF in f64, scale by 16 and quantize to
    e4m3, pre-transpose into DoubleRow-blocked layouts, compute diag sims
    and the pre-feature cosine term (both O(N*D), same class as the
    normalization already done here).
  - Core (rg, cg) of a 2x4 grid computes its [2048, 1024] tile of
    E = exp(Vn @ An^T):
      * TensorE: 16 groups x 8 fp8 DoubleRow matmuls (K=256 each) into a
        [128, 1024] PSUM pair; short HAM-warmup matmuls first so the clock
        ramp overlaps the initial DMA wait
      * ScalarE: exp(PSUM / 256) with fused row-sum (accum_out)
      * VectorE: f32 column-partial adds; the last group is split into
        512-col halves so the exp->add->store tail is half as deep
      * DMA scheduling (trace-derived): each HWDGE queue retires ~90
        descriptors/us no matter the line size, so every dma_start moves
        128 lines in ~1.4us — use the fattest lines possible; sync's
        queue starts ~1us before scalar's; same-tile DMAs from two
        engines serialize (coarse WAW), so vn is split into two SBUF
        tiles homed on different engines.
    Outputs per core: rowsum partials [128, 17] (last chunk split in
    halves), column partials [128, 1024] bf16 (partition-summed on host).
  - Host: O(N) final assembly (log/ratio/sums) in f64.
"""

import os
import sys

import numpy as np

for _p in ("/opt/trn_rl_repo",):
    if _p not in sys.path and os.path.isdir(_p):
        sys.path.insert(0, _p)

N = 4096
D = 1024
NCORES = 8
RG = 2                   # row groups
CG = 4                   # col groups
ROWS = N // RG           # 2048 rows per core
COLS = N // CG           # 1024 cols per core
MCH = ROWS // 128        # 16 row chunks per core
MCL = MCH // 2           # 8 chunks per vn half-tile
KCH = D // 128           # 8 contraction chunks of 128
KD2 = KCH // 2           # 4 DoubleRow chunks of 256
NB = 512                 # matmul moving free dim (one PSUM bank)
NBL = COLS // NB         # 2 column blocks per core

MARGIN = 0.2
BALANCE = 0.5
BIAS = 1.0
EPS = 1e-18

FP8_SCALE = 16.0  # host pre-scale so e4m3 keeps the values out of subnormals

_CACHE = {}
LAST_RESULT = None  # BassKernelResults of the most recent run (for test harness)


def _build_nc():
    import concourse.bass as bass  # noqa: F401
    import concourse.bacc as bacc
    import concourse.tile as tile
    from concourse import mybir
    from contextlib import ExitStack

    BF16 = mybir.dt.bfloat16
    F32 = mybir.dt.float32
    FP8 = mybir.dt.float8e4
    Exp = mybir.ActivationFunctionType.Exp
    DoubleRow = mybir.MatmulPerfMode.DoubleRow

    nc = bacc.Bacc("TRN2", debug=False, num_devices=NCORES)

    MCW = 1024  # vnT columns per row chunk
    HALF = MCL * MCW  # 8192 columns per vn half

    # DRAM I/O (per core). Layouts chosen so every DMA is contiguous.
    # vnT[p, mc*1024 + k2*256 + i*128 + dm] = Vn_slab[mc*128 + dm, (2*k2+i)*128 + p] * FP8_SCALE
    vnT_d = nc.dram_tensor("vnT", [128, MCH * MCW], FP8, kind="ExternalInput")
    # anT[n, p, k2*2*NB + i*NB + c] = An_slab[n*NB + c, (2*k2+i)*128 + p] * FP8_SCALE
    anT_d = nc.dram_tensor("anT", [NBL, 128, KCH * NB], FP8, kind="ExternalInput")

    # rowsum[p, mc] partials; the last chunk is split into columns MCH-1, MCH
    rowsum_d = nc.dram_tensor("rowsum", [128, MCH + 1], F32, kind="ExternalOutput")
    # colp[p, j] = sum over mc of exp chunk [mc][p, j]  (host sums partitions)
    colp_d = nc.dram_tensor("colp", [128, COLS], BF16, kind="ExternalOutput")

    with tile.TileContext(nc) as tc:
        with ExitStack() as ctx:
            singles = ctx.enter_context(tc.tile_pool(name="singles", bufs=1))

            vn_t0 = singles.tile([128, 2 * MCW], FP8, tag="vn_t0")
            vn_t1 = singles.tile([128, HALF - 2 * MCW], FP8, tag="vn_t1")
            vn_hi = singles.tile([128, HALF], FP8, tag="vn_hi")
            an_sb = []
            for n in range(NBL):
                an_t = singles.tile([128, KCH * NB], FP8, tag=f"an{n}")
                an_sb.append(an_t)

            # DMA schedule. Queues retire ~80-90 descriptors/us and every
            # dma_start is 128 partition lines, so each start costs ~1.4us
            # of queue time; concurrent queues steal each other's
            # descriptor slots. So: ALL input DMAs ride sync's queue
            # (starts ~8.7us), strictly in stream-consumption order —
            # vn_t0 ~10.2, an0 ~11.6 (first matmul), an1 ~13.1 (needed
            # ~13.3), vn_t1 ~15.2 (needed ~18), vn_hi ~17.5 (needed ~19.5).
            # Scalar's queue stays empty so it can't contend.
            nc.sync.dma_start(vn_t0[:], vnT_d.ap()[:, 0 : 2 * MCW])
            nc.sync.dma_start(an_sb[0][:], anT_d.ap()[0])
            nc.sync.dma_start(an_sb[1][:], anT_d.ap()[1])
            nc.sync.dma_start(vn_t1[:], vnT_d.ap()[:, 2 * MCW : HALF])
            nc.sync.dma_start(vn_hi[:], vnT_d.ap()[:, HALF : 2 * HALF])

            efold = singles.tile([128, COLS], F32, tag="efold")
            colp = singles.tile([128, COLS], BF16, tag="colp")
            rs = singles.tile([128, MCH + 1], F32, tag="rs")
            ones_b = singles.tile([128, 1], BF16, tag="ones_b")
            nc.vector.memset(ones_b[:], 1.0)
            dummy = singles.tile([128, 256], BF16, tag="dummy")
            nc.vector.memset(dummy[:], 0.0)

            psum = ctx.enter_context(tc.tile_pool(name="mm_psum", bufs=3, space="PSUM"))
            wup = ctx.enter_context(tc.tile_pool(name="wup_psum", bufs=1, space="PSUM"))
            epool = ctx.enter_context(tc.tile_pool(name="etile", bufs=3))

            # HAM warmup: keep TensorE busy during the initial DMA wait so
            # the clock ramp starts as early as possible.
            wps = wup.tile([128, 256], mybir.dt.float32, tag="wup")
            NWARM = 16
            for i in range(NWARM):
                nc.tensor.matmul(
                    wps[0:1, :], ones_b[:], dummy[:],
                    start=(i == 0), stop=(i == NWARM - 1),
                )

            # Main stream: 16 groups of 8 DoubleRow matmuls. Half-outer:
            # group 0's first four matmuls touch only an0, buying an1 an
            # extra ~1.7us of DMA time; the last group's halves retire
            # ~0.85us apart, shortening the exp->add->store tail.
            descale = 1.0 / (FP8_SCALE * FP8_SCALE)
            for mc in range(MCH):
                if mc < 2:
                    vtile, voff = vn_t0, mc * MCW
                elif mc < MCL:
                    vtile, voff = vn_t1, (mc - 2) * MCW
                else:
                    vtile, voff = vn_hi, (mc - MCL) * MCW
                ps = psum.tile([128, 2 * NB], mybir.dt.float32)
                for half in range(NBL):
                    a_t = an_sb[half]
                    for k2 in range(KD2):
                        w3 = (
                            vtile[:, voff + k2 * 256 : voff + (k2 + 1) * 256]
                            .rearrange("p (i m) -> p i m", i=2)
                        )
                        a3 = (
                            a_t[:, k2 * 2 * NB : (k2 + 1) * 2 * NB]
                            .rearrange("p (i c) -> p i c", i=2)
                        )
                        nc.tensor.matmul(
                            ps[:, half * NB : (half + 1) * NB],
                            w3,
                            a3,
                            start=(k2 == 0),
                            stop=(k2 == KD2 - 1),
                            perf_mode=DoubleRow,
                        )
                if mc == 0:
                    nc.scalar.activation(
                        efold[:], ps[:], Exp, scale=descale,
                        accum_out=rs[:, mc : mc + 1],
                    )
                elif mc == MCH - 1:
                    # Last chunk in halves: half the serial exp->add->store
                    # tail. Row-sum halves land in rs cols MCH-1 / MCH
                    # (host adds); per-half colp DMA overlaps the other
                    # half's compute.
                    et = epool.tile([128, 2 * NB], F32)
                    for h in range(2):
                        hs = slice(h * NB, (h + 1) * NB)
                        nc.scalar.activation(
                            et[:, hs], ps[:, hs], Exp, scale=descale,
                            accum_out=rs[:, mc + h : mc + h + 1],
                        )
                        nc.vector.tensor_add(colp[:, hs], efold[:, hs], et[:, hs])
                        nc.sync.dma_start(colp_d.ap()[:, hs], colp[:, hs])
                    # rowsum rides scalar's otherwise-idle queue, right
                    # after its last accumulator read
                    nc.scalar.dma_start(rowsum_d.ap(), rs[:])
                else:
                    et = epool.tile([128, 2 * NB], F32)
                    nc.scalar.activation(
                        et[:], ps[:], Exp, scale=descale,
                        accum_out=rs[:, mc : mc + 1],
                    )
                    nc.vector.tensor_add(efold[:], efold[:], et[:])

    nc.compile()
    return nc


def _get_nc():
    if "nc" not in _CACHE:
        _CACHE["nc"] = _build_nc()
    return _CACHE["nc"]


def _prep_inputs(pre_VF, pre_AF, back_VF, back_AF):
    """Normalize + quantize + relayout on host; returns per-core in_maps,
    host diag, and the host pre-cosine term."""
    import ml_dtypes

    V = np.asarray(back_VF, dtype=np.float64)
    A = np.asarray(back_AF, dtype=np.float64)
    Vn = V / np.sqrt((V * V).sum(-1, keepdims=True) + EPS)
    An = A / np.sqrt((A * A).sum(-1, keepdims=True) + EPS)
    diag = np.einsum("ij,ij->i", Vn, An)

    pV = np.asarray(pre_VF, dtype=np.float64)
    pA = np.asarray(pre_AF, dtype=np.float64)
    pVn = pV / np.sqrt((pV * pV).sum(-1, keepdims=True) + EPS)
    pAn = pA / np.sqrt((pA * pA).sum(-1, keepdims=True) + EPS)
    pre_cos = np.einsum("ij,ij->i", pVn, pAn)

    fp8 = ml_dtypes.float8_e4m3
    Vn8 = (Vn * FP8_SCALE).astype(fp8)
    An8 = (An * FP8_SCALE).astype(fp8)

    # vnT[p, mc*1024 + k2*256 + i*128 + dm] = Vn8_slab[mc*128 + dm, (2*k2+i)*128 + p]
    vnTs = []
    for rg in range(RG):
        sl = Vn8[rg * ROWS : (rg + 1) * ROWS]
        vnTs.append(
            np.ascontiguousarray(
                sl.reshape(MCH, 128, KD2, 2, 128)  # [mc, dm, k2, i, p]
                .transpose(4, 0, 2, 3, 1)          # [p, mc, k2, i, dm]
                .reshape(128, MCH * KD2 * 2 * 128)
            )
        )

    # anT[n, p, k2*2*NB + i*NB + c] = An8_slab[n*NB + c, (2*k2+i)*128 + p]
    anTs = []
    for cg in range(CG):
        sl = An8[cg * COLS : (cg + 1) * COLS]
        anTs.append(
            np.ascontiguousarray(
                sl.reshape(NBL, NB, KD2, 2, 128)   # [n, c, k2, i, p]
                .transpose(0, 4, 2, 3, 1)          # [n, p, k2, i, c]
                .reshape(NBL, 128, KCH * NB)
            )
        )

    in_maps = []
    for c in range(NCORES):
        rg, cg = c // CG, c % CG
        in_maps.append({"vnT": vnTs[rg], "anT": anTs[cg]})
    return in_maps, diag, pre_cos


def _assemble(outs, diag, pre_cos):
    """O(N) final reduction on host, f64."""
    rowsum = np.zeros(N, dtype=np.float64)
    colsum = np.zeros(N, dtype=np.float64)
    for c in range(NCORES):
        rg, cg = c // CG, c % CG
        # rowsum[p, mc]: row = rg*ROWS + mc*128 + p, partial over this cg;
        # the last chunk's sum is split across columns MCH-1 and MCH
        rsc = outs[c]["rowsum"].astype(np.float64)  # [128, MCH + 1]
        rsc = np.concatenate(
            [rsc[:, : MCH - 1], (rsc[:, MCH - 1] + rsc[:, MCH])[:, None]], axis=1
        )
        rowsum[rg * ROWS : (rg + 1) * ROWS] += rsc.T.reshape(ROWS)
        colsum[cg * COLS : (cg + 1) * COLS] += (
            outs[c]["colp"].astype(np.float64).sum(axis=0)
        )

    dE = np.exp(diag)
    pos = np.exp(diag - MARGIN)
    neg_V = rowsum - dE
    neg_A = colsum - dE
    L_V = np.log(pos / (pos + neg_V)).sum()
    L_A = np.log(pos / (pos + neg_A)).sum()
    L_pre = pre_cos.sum()

    loss = BALANCE * (-1.0 / BIAS) * (L_V + L_A) + (1.0 - BALANCE) * L_pre
    return np.array(loss, dtype=np.float32)


def kernel(pre_VF, pre_AF, back_VF, back_AF):
    global LAST_RESULT
    from concourse import bass_utils

    nc = _get_nc()
    in_maps, diag, pre_cos = _prep_inputs(pre_VF, pre_AF, back_VF, back_AF)
    res = bass_utils.run_bass_kernel_spmd(nc, in_maps, core_ids=list(range(NCORES)))
    LAST_RESULT = res
    return _assemble(res.results, diag, pre_cos)
